# revision 16
# baseline (speedup 1.0000x reference)
"""Trainium2 Bass kernel for grouped 128x128 sparse attention + output proj.

Problem (hardcoded): qkv [2, 65536, 576] f32, tk_id [2, 65536] int32 in [0,64),
proj_w [192,192], proj_b [192].  c=192, heads=6, dh=32, group size 128,
ng=512 per batch (no padding since 65536 % 128 == 0).

Strategy:
  - Host: stable argsort by cluster id, gather qkv, reshape to 1024 independent
    groups; data-parallel shard 128 groups/core across 8 NeuronCores.
  - Device, per pair of groups (two groups share the 128-token tiles):
      scores: 12 K=32 matmuls row-tiled 2x via tile_position=(32i,0) into
        4 PSUM banks (two [128,1024] tiles, one per group j)
      exp:    2 scalar activations (FD=768 each) -> SBUF bf16
      AV:     12 matmuls P^T.T @ [v|1] -> av [q, 12*33] f32 (col 33h+32 = denom)
      normalize: vector reciprocal + one broadcast multiply -> att bf16
      transpose: 4 PE transposes into the proj psum bank (bf16 bitcast view)
      proj:   4 matmuls (aT.T @ wt1/wt2, bias via exact-1.0 denom rows)
      out:    vector copy psum -> SBUF bf16, DMA per 4-pair chunk
  - Host: gather per-core outputs, inverse permutation, return [2, 65536, 192].

All DMAs move 4-pair chunks to amortize descriptor-issue cost.
"""

import numpy as np
import ml_dtypes

BF16 = ml_dtypes.bfloat16

B = 2
N = 65536
C = 192
H = 6
DH = 32
GS = 128
NG_TOTAL = B * (N // GS)  # 1024 groups
N_CORES = 8
G_PER_CORE = NG_TOTAL // N_CORES  # 128
PAIRS = G_PER_CORE // 2  # 64
CHUNK = 4  # pairs per DMA chunk
NCHUNK = PAIRS // CHUNK  # 16

_nc_cache = {}


def _build_nc(num_pairs=PAIRS):
    """Build the Bass/Tile graph for one core (SPMD across all 8)."""
    from contextlib import ExitStack

    import concourse.tile as tile
    from concourse import bacc, mybir

    bf = mybir.dt.bfloat16
    f32 = mybir.dt.float32
    EXPF = mybir.ActivationFunctionType.Exp

    nc = bacc.Bacc("TRN2", target_bir_lowering=False, debug=False)

    P = num_pairs
    NCH = P // CHUNK
    # qk: per pair [32 partitions, 3072 cols]; head m=6j+h:
    #   q_m at cols 256m:256m+128, k_m at 256m+128:256m+256.  Matmuls run
    #   K=128 with partitions 32:128 zeroed once (full-array MMs keep the
    #   PE HAM state warm; tiled K=32 MMs don't count as PE-busy and left
    #   the whole PE throttled at half rate).
    qks_d = nc.declare_dram_parameter("qks", [NCH, 32, CHUNK * 3072], bf, isOutput=False)
    # v1: per pair [128, 396]; cols 198j+33h+[v(32)|1].
    v1_d = nc.declare_dram_parameter("v1", [NCH, 128, CHUNK * 396], bf, isOutput=False)
    wt1_d = nc.declare_dram_parameter("wt1", [128, 192], bf, isOutput=False)
    wt2_d = nc.declare_dram_parameter("wt2", [128, 192], bf, isOutput=False)
    iden_d = nc.declare_dram_parameter("iden", [128, 128], bf, isOutput=False)
    out_d = nc.declare_dram_parameter("out", [NCH, 128, CHUNK * 384], bf, isOutput=True)

    with tile.TileContext(nc) as tc, ExitStack() as ctx:
        consts = ctx.enter_context(tc.tile_pool(name="consts", bufs=1))
        wt1_sb = consts.tile([128, 192], bf)
        nc.sync.dma_start(out=wt1_sb[:], in_=wt1_d[:, :])
        wt2_sb = consts.tile([128, 192], bf)
        nc.sync.dma_start(out=wt2_sb[:], in_=wt2_d[:, :])
        iden_sb = consts.tile([128, 128], bf)
        nc.sync.dma_start(out=iden_sb[:], in_=iden_d[:, :])

        qkbufs = [consts.tile([128, CHUNK * 3072], bf, name=f"qkc{i}") for i in range(3)]
        for qb in qkbufs:
            nc.vector.memset(qb[:], 0.0)
        vvbufs = [consts.tile([128, CHUNK * 396], bf, name=f"vvc{i}") for i in range(3)]
        obbufs = [consts.tile([128, CHUNK * 384], bf, name=f"obc{i}") for i in range(3)]
        # att: [q, 256j + 33h + d], cols 198:256 / 454:512 stay zero (pad)
        attbufs = [consts.tile([128, 512], bf, name=f"attb{i}") for i in range(3)]
        for ab in attbufs:
            nc.vector.memset(ab[:, 198:256], 0.0)
            nc.vector.memset(ab[:, 454:512], 0.0)
        aTbufs = [consts.tile([128, 512], bf, name=f"aTb{i}") for i in range(3)]

        expp = ctx.enter_context(tc.tile_pool(name="exps", bufs=4))
        recp = ctx.enter_context(tc.tile_pool(name="rec", bufs=2))
        # PSUM: gt 2x2 banks + av 2 + pj 2 = 8 banks.
        gtp = ctx.enter_context(tc.tile_pool(name="gt", bufs=2, space="PSUM"))
        avp = ctx.enter_context(tc.tile_pool(name="av", bufs=2, space="PSUM"))
        pjp = ctx.enter_context(tc.tile_pool(name="pj", bufs=2, space="PSUM"))

        # chunk 0: per-pair qk slices so pair 0 can start before the rest land
        for r in range(CHUNK):
            nc.sync.dma_start(
                out=qkbufs[0][0:32, 3072 * r : 3072 * (r + 1)],
                in_=qks_d[0, :, 3072 * r : 3072 * (r + 1)],
            )
        nc.sync.dma_start(out=vvbufs[0][:], in_=v1_d[0])
        nc.sync.dma_start(out=qkbufs[1][0:32, :], in_=qks_d[1])
        nc.sync.dma_start(out=vvbufs[1][:], in_=v1_d[1])

        e_tiles = [None] * P
        att_map = [None] * P
        pj_tiles = [None] * P

        for it in range(P + 3):
            # Prefetch 2 chunks ahead into the 3-deep ring: by it%CHUNK==1 the
            # last readers of buffer k%3 (scores/AV of chunk k-3) are already
            # emitted, so the WAR dep is inferred correctly, and the ~5us
            # chunk transfer has two chunk-periods to complete.
            if it < P and it % CHUNK == 1:
                k = it // CHUNK + 2
                if k < NCH:
                    nc.sync.dma_start(out=qkbufs[k % 3][0:32, :], in_=qks_d[k])
                    nc.sync.dma_start(out=vvbufs[k % 3][:], in_=v1_d[k])

            if it < P:
                # scores + exp for pair `it`
                p = it
                qk = qkbufs[(p // CHUNK) % 3]
                qo = 3072 * (p % CHUNK)
                gts = [
                    gtp.tile([128, 1024], f32, tag="gt", name=f"gt{p}_{j}")
                    for j in range(2)
                ]
                for m in range(12):
                    gt = gts[m // 6]
                    h = m % 6
                    nc.tensor.matmul(
                        gt[:, 128 * h : 128 * h + 128],
                        qk[0:128, qo + 256 * m + 128 : qo + 256 * m + 256],
                        qk[0:128, qo + 256 * m : qo + 256 * m + 128],
                        start=True,
                        stop=True,
                    )
                ea = expp.tile([128, 768], bf, tag="exp", name=f"ea{p}")
                eb = expp.tile([128, 768], bf, tag="exp", name=f"eb{p}")
                nc.scalar.activation(ea[:], gts[0][:, 0:768], EXPF)
                nc.scalar.activation(eb[:], gts[1][:, 0:768], EXPF)
                e_tiles[p] = (ea, eb)

            if 1 <= it <= P:
                # AV + normalize for pair it-1
                p = it - 1
                ea, eb = e_tiles[p]
                e_tiles[p] = None
                vv = vvbufs[(p // CHUNK) % 3]
                vo = 396 * (p % CHUNK)
                av = avp.tile([128, 396], f32, tag="av", name=f"av{p}")
                for m in range(12):
                    j, h = m // 6, m % 6
                    e = ea if j == 0 else eb
                    nc.tensor.matmul(
                        av[:, 198 * j + 33 * h : 198 * j + 33 * h + 33],
                        e[:, 128 * h : 128 * h + 128],
                        vv[:, vo + 198 * j + 33 * h : vo + 198 * j + 33 * h + 33],
                        start=True,
                        stop=True,
                    )
                av4 = av[:].rearrange("p (j h x) -> p j h x", j=2, x=33)
                rec = recp.tile([128, 12], f32)
                rec3 = rec[:].rearrange("p (j h) -> p j h", j=2)
                nc.vector.reciprocal(rec3[:, :, :, None], av4[:, :, :, 32:33])
                att = attbufs[p % 3]
                attv = (
                    att[:]
                    .rearrange("p (j x) -> p j x", j=2)[:, :, 0:198]
                    .rearrange("p j (h d) -> p j h d", d=33)
                )
                nc.vector.tensor_mul(
                    attv, av4, rec3[:, :, :, None].to_broadcast((128, 2, 6, 33))
                )
                att_map[p] = att

            if 2 <= it <= P + 1:
                # transpose + copy to SBUF for pair it-2
                p = it - 2
                att = att_map[p]
                att_map[p] = None
                pjt = pjp.tile([128, 384], f32, tag="pj", name=f"pj{p}")
                tp = pjt[:, 0:256].bitcast(bf)  # [128, 512] bf16 staging
                for j in range(2):
                    nc.tensor.transpose(
                        tp[:, 128 * j : 128 * j + 128],
                        att[:, 256 * j : 256 * j + 128],
                        iden_sb[:],
                    )
                    nc.tensor.transpose(
                        tp[:, 256 + 128 * j : 256 + 128 * j + 128],
                        att[:, 256 * j + 128 : 256 * j + 256],
                        iden_sb[:],
                    )
                aT = aTbufs[p % 3]
                nc.vector.tensor_copy(aT[:], tp[:])
                pj_tiles[p] = (pjt, aT)

            if it >= 3:
                # proj + output copy for pair it-3
                p = it - 3
                pjt, aT = pj_tiles[p]
                pj_tiles[p] = None
                for j in range(2):
                    pj = pjt[:, 192 * j : 192 * j + 192]
                    nc.tensor.matmul(
                        pj,
                        aT[:, 128 * j : 128 * j + 128],
                        wt1_sb[:],
                        start=True,
                        stop=False,
                    )
                    nc.tensor.matmul(
                        pj,
                        aT[:, 256 + 128 * j : 256 + 128 * j + 128],
                        wt2_sb[:],
                        start=False,
                        stop=True,
                    )
                ob = obbufs[(p // CHUNK) % 3]
                r = p % CHUNK
                nc.vector.tensor_copy(ob[:, 384 * r : 384 * r + 384], pjt[:])
                if r == CHUNK - 1:
                    nc.gpsimd.dma_start(out=out_d[p // CHUNK], in_=ob[:])

    nc.compile()
    return nc


def _host_prep(qkv, tk_id, proj_w, proj_b):
    """Sort/gather/layout on host. Returns (in_maps, sort_idx)."""
    qkv = np.asarray(qkv, dtype=np.float32)
    tk_id = np.asarray(tk_id)
    proj_w = np.asarray(proj_w, dtype=np.float32)
    proj_b = np.asarray(proj_b, dtype=np.float32)

    sort_idx = np.argsort(tk_id, axis=-1, kind="stable")  # [B, N]
    shuffled = np.take_along_axis(qkv, sort_idx[:, :, None], axis=1)  # [B,N,3C]

    y = shuffled.reshape(B, N // GS, GS, 3, H, DH).reshape(NG_TOTAL, GS, 3, H, DH)
    scale = DH ** (-0.5)
    q = y[:, :, 0] * scale  # [G, t, h, d]
    k = y[:, :, 1]
    v = y[:, :, 2]

    Ptot = NG_TOTAL // 2  # 512
    qp = q.reshape(Ptot, 2, GS, H, DH)  # [p, j, t, h, d]
    kp = k.reshape(Ptot, 2, GS, H, DH)
    qk = np.empty((Ptot, 32, 3072), dtype=BF16)
    for m in range(12):
        j, h = m // 6, m % 6
        qk[:, :, 256 * m : 256 * m + 128] = qp[:, j, :, h, :].transpose(0, 2, 1)
        qk[:, :, 256 * m + 128 : 256 * m + 256] = kp[:, j, :, h, :].transpose(0, 2, 1)

    v1 = np.empty((NG_TOTAL, GS, H, DH + 1), dtype=np.float32)
    v1[..., :DH] = v
    v1[..., DH] = 1.0
    v1 = v1.reshape(NG_TOTAL, GS, H * (DH + 1))  # [G, 128, 198]
    v1p = (
        v1.reshape(Ptot, 2, GS, 198)
        .transpose(0, 2, 1, 3)
        .reshape(Ptot, GS, 396)
        .astype(BF16)
    )

    # proj weights permuted to att-column order (33h+d; d==32 -> bias/6 row)
    wt = proj_w.T.copy()  # [cin, cout]
    b6 = proj_b / 6.0
    wt1 = np.zeros((128, C), np.float32)
    wt2 = np.zeros((128, C), np.float32)
    for r in range(128):
        h, d = r // 33, r % 33
        wt1[r] = b6 if d == 32 else wt[32 * h + d]
    for rp in range(70):
        col = 128 + rp
        h, d = col // 33, col % 33
        wt2[rp] = b6 if d == 32 else wt[32 * h + d]
    wt1 = wt1.astype(BF16)
    wt2 = wt2.astype(BF16)
    iden = np.eye(128, dtype=BF16)

    in_maps = []
    for core in range(N_CORES):
        s = slice(core * PAIRS, (core + 1) * PAIRS)
        qkc = (
            qk[s]
            .reshape(NCHUNK, CHUNK, 32, 3072)
            .transpose(0, 2, 1, 3)
            .reshape(NCHUNK, 32, CHUNK * 3072)
        )
        v1c = (
            v1p[s]
            .reshape(NCHUNK, CHUNK, 128, 396)
            .transpose(0, 2, 1, 3)
            .reshape(NCHUNK, 128, CHUNK * 396)
        )
        in_maps.append(
            {
                "qks": np.ascontiguousarray(qkc),
                "v1": np.ascontiguousarray(v1c),
                "wt1": wt1,
                "wt2": wt2,
                "iden": iden,
            }
        )
    return in_maps, sort_idx


def _host_unshard(results, sort_idx):
    outs = []
    for res in results:
        o = np.asarray(res["out"])  # [NCHUNK, 128, CHUNK*384] bf16
        o = (
            o.reshape(NCHUNK, 128, CHUNK, 384)
            .transpose(0, 2, 1, 3)
            .reshape(PAIRS, 128, 384)
        )
        outs.append(o)
    out_sorted = np.concatenate(outs, axis=0).astype(np.float32)  # [512, 128, 384]
    out_sorted = (
        out_sorted.reshape(NG_TOTAL // 2, GS, 2, C)
        .transpose(0, 2, 1, 3)
        .reshape(B, N, C)
    )
    final = np.empty_like(out_sorted)
    np.put_along_axis(final, sort_idx[:, :, None], out_sorted, axis=1)
    return final


def _get_nc():
    if "nc" not in _nc_cache:
        _nc_cache["nc"] = _build_nc()
    return _nc_cache["nc"]


def _run(in_maps, trace=False):
    from concourse import bass_utils

    nc = _get_nc()
    return bass_utils.run_bass_kernel_spmd(
        nc, in_maps, core_ids=list(range(N_CORES)), trace=trace
    )


def kernel(qkv, tk_id, x_size=None, proj_w=None, proj_b=None):
    in_maps, sort_idx = _host_prep(qkv, tk_id, proj_w, proj_b)
    res = _run(in_maps, trace=False)
    return _host_unshard(res.results, sort_idx)


# revision 17
# speedup vs baseline: 1.0089x; 1.0089x over previous
"""Trainium2 Bass kernel for grouped 128x128 sparse attention + output proj.

Problem (hardcoded): qkv [2, 65536, 576] f32, tk_id [2, 65536] int32 in [0,64),
proj_w [192,192], proj_b [192].  c=192, heads=6, dh=32, group size 128,
ng=512 per batch (no padding since 65536 % 128 == 0).

Strategy:
  - Host: stable argsort by cluster id, gather qkv, reshape to 1024 independent
    groups; data-parallel shard 128 groups/core across 8 NeuronCores.
  - Device, per pair of groups (two groups share the 128-token tiles):
      scores: 12 K=32 matmuls row-tiled 2x via tile_position=(32i,0) into
        4 PSUM banks (two [128,1024] tiles, one per group j)
      exp:    2 scalar activations (FD=768 each) -> SBUF bf16
      AV:     12 matmuls P^T.T @ [v|1] -> av [q, 12*33] f32 (col 33h+32 = denom)
      normalize: vector reciprocal + one broadcast multiply -> att bf16
      transpose: 4 PE transposes into the proj psum bank (bf16 bitcast view)
      proj:   4 matmuls (aT.T @ wt1/wt2, bias via exact-1.0 denom rows)
      out:    vector copy psum -> SBUF bf16, DMA per 4-pair chunk
  - Host: gather per-core outputs, inverse permutation, return [2, 65536, 192].

All DMAs move 4-pair chunks to amortize descriptor-issue cost.
"""

import numpy as np
import ml_dtypes

BF16 = ml_dtypes.bfloat16

B = 2
N = 65536
C = 192
H = 6
DH = 32
GS = 128
NG_TOTAL = B * (N // GS)  # 1024 groups
N_CORES = 8
G_PER_CORE = NG_TOTAL // N_CORES  # 128
PAIRS = G_PER_CORE // 2  # 64
CHUNK = 4  # pairs per DMA chunk
NCHUNK = PAIRS // CHUNK  # 16

_nc_cache = {}


def _build_nc(num_pairs=PAIRS):
    """Build the Bass/Tile graph for one core (SPMD across all 8)."""
    from contextlib import ExitStack

    import concourse.tile as tile
    from concourse import bacc, mybir

    bf = mybir.dt.bfloat16
    f32 = mybir.dt.float32
    EXPF = mybir.ActivationFunctionType.Exp

    nc = bacc.Bacc("TRN2", target_bir_lowering=False, debug=False)

    P = num_pairs
    NCH = P // CHUNK
    # qk: per pair [32 partitions, 3072 cols]; head m=6j+h:
    #   q_m at cols 256m:256m+128, k_m at 256m+128:256m+256.  Matmuls run
    #   K=128 with partitions 32:128 zeroed once (full-array MMs keep the
    #   PE HAM state warm; tiled K=32 MMs don't count as PE-busy and left
    #   the whole PE throttled at half rate).
    qks_d = nc.declare_dram_parameter("qks", [NCH, 32, CHUNK * 3072], bf, isOutput=False)
    # v1: per pair [128, 396]; cols 198j+33h+[v(32)|1].
    v1_d = nc.declare_dram_parameter("v1", [NCH, 128, CHUNK * 396], bf, isOutput=False)
    wt1_d = nc.declare_dram_parameter("wt1", [128, 192], bf, isOutput=False)
    wt2_d = nc.declare_dram_parameter("wt2", [128, 192], bf, isOutput=False)
    iden_d = nc.declare_dram_parameter("iden", [128, 128], bf, isOutput=False)
    out_d = nc.declare_dram_parameter("out", [NCH, 128, CHUNK * 384], bf, isOutput=True)

    with tile.TileContext(nc) as tc, ExitStack() as ctx:
        consts = ctx.enter_context(tc.tile_pool(name="consts", bufs=1))
        wt1_sb = consts.tile([128, 192], bf)
        nc.sync.dma_start(out=wt1_sb[:], in_=wt1_d[:, :])
        wt2_sb = consts.tile([128, 192], bf)
        nc.sync.dma_start(out=wt2_sb[:], in_=wt2_d[:, :])
        iden_sb = consts.tile([128, 128], bf)
        nc.sync.dma_start(out=iden_sb[:], in_=iden_d[:, :])

        qkbufs = [consts.tile([128, CHUNK * 3072], bf, name=f"qkc{i}") for i in range(3)]
        for qb in qkbufs:
            nc.vector.memset(qb[:], 0.0)
        vvbufs = [consts.tile([128, CHUNK * 396], bf, name=f"vvc{i}") for i in range(3)]
        obbufs = [consts.tile([128, CHUNK * 384], bf, name=f"obc{i}") for i in range(3)]
        # att: [q, 256j + 33h + d], cols 198:256 / 454:512 stay zero (pad)
        attbufs = [consts.tile([128, 512], bf, name=f"attb{i}") for i in range(3)]
        for ab in attbufs:
            nc.vector.memset(ab[:, 198:256], 0.0)
            nc.vector.memset(ab[:, 454:512], 0.0)
        aTbufs = [consts.tile([128, 512], bf, name=f"aTb{i}") for i in range(3)]

        expp = ctx.enter_context(tc.tile_pool(name="exps", bufs=4))
        recp = ctx.enter_context(tc.tile_pool(name="rec", bufs=2))
        # PSUM: gt 2x2 banks + av 2 + pj 2 = 8 banks.
        gtp = ctx.enter_context(tc.tile_pool(name="gt", bufs=2, space="PSUM"))
        avp = ctx.enter_context(tc.tile_pool(name="av", bufs=2, space="PSUM"))
        pjp = ctx.enter_context(tc.tile_pool(name="pj", bufs=2, space="PSUM"))

        # chunk 0: per-pair qk slices so pair 0 can start before the rest land
        for r in range(CHUNK):
            nc.sync.dma_start(
                out=qkbufs[0][0:32, 3072 * r : 3072 * (r + 1)],
                in_=qks_d[0, :, 3072 * r : 3072 * (r + 1)],
            )
        nc.sync.dma_start(out=vvbufs[0][:], in_=v1_d[0])
        nc.sync.dma_start(out=qkbufs[1][0:32, :], in_=qks_d[1])
        nc.sync.dma_start(out=vvbufs[1][:], in_=v1_d[1])

        e_tiles = [None] * P
        att_map = [None] * P
        pj_tiles = [None] * P

        for it in range(P + 3):
            # Prefetch 2 chunks ahead into the 3-deep ring: by it%CHUNK==1 the
            # last readers of buffer k%3 (scores/AV of chunk k-3) are already
            # emitted, so the WAR dep is inferred correctly, and the ~5us
            # chunk transfer has two chunk-periods to complete.
            if it < P and it % CHUNK == 1:
                k = it // CHUNK + 2
                if k < NCH:
                    nc.sync.dma_start(out=qkbufs[k % 3][0:32, :], in_=qks_d[k])
                    nc.sync.dma_start(out=vvbufs[k % 3][:], in_=v1_d[k])

            if it < P:
                # scores + exp for pair `it`.  high_priority keeps the
                # scores -> EXP feed ahead of older tail work in the static
                # schedule so the scalar engine (the pacing engine) never
                # starves behind a transpose/proj backlog.
                p = it
                qk = qkbufs[(p // CHUNK) % 3]
                qo = 3072 * (p % CHUNK)
                gts = [
                    gtp.tile([128, 1024], f32, tag="gt", name=f"gt{p}_{j}")
                    for j in range(2)
                ]
                with tc.high_priority(offset=200):
                    for m in range(12):
                        gt = gts[m // 6]
                        h = m % 6
                        nc.tensor.matmul(
                            gt[:, 128 * h : 128 * h + 128],
                            qk[0:128, qo + 256 * m + 128 : qo + 256 * m + 256],
                            qk[0:128, qo + 256 * m : qo + 256 * m + 128],
                            start=True,
                            stop=True,
                        )
                    ea = expp.tile([128, 768], bf, tag="exp", name=f"ea{p}")
                    eb = expp.tile([128, 768], bf, tag="exp", name=f"eb{p}")
                    nc.scalar.activation(ea[:], gts[0][:, 0:768], EXPF)
                    nc.scalar.activation(eb[:], gts[1][:, 0:768], EXPF)
                e_tiles[p] = (ea, eb)

            if 1 <= it <= P:
                # AV + normalize for pair it-1
                p = it - 1
                ea, eb = e_tiles[p]
                e_tiles[p] = None
                vv = vvbufs[(p // CHUNK) % 3]
                vo = 396 * (p % CHUNK)
                av = avp.tile([128, 396], f32, tag="av", name=f"av{p}")
                for m in range(12):
                    j, h = m // 6, m % 6
                    e = ea if j == 0 else eb
                    nc.tensor.matmul(
                        av[:, 198 * j + 33 * h : 198 * j + 33 * h + 33],
                        e[:, 128 * h : 128 * h + 128],
                        vv[:, vo + 198 * j + 33 * h : vo + 198 * j + 33 * h + 33],
                        start=True,
                        stop=True,
                    )
                av4 = av[:].rearrange("p (j h x) -> p j h x", j=2, x=33)
                rec = recp.tile([128, 12], f32)
                rec3 = rec[:].rearrange("p (j h) -> p j h", j=2)
                nc.vector.reciprocal(rec3[:, :, :, None], av4[:, :, :, 32:33])
                att = attbufs[p % 3]
                attv = (
                    att[:]
                    .rearrange("p (j x) -> p j x", j=2)[:, :, 0:198]
                    .rearrange("p j (h d) -> p j h d", d=33)
                )
                nc.vector.tensor_mul(
                    attv, av4, rec3[:, :, :, None].to_broadcast((128, 2, 6, 33))
                )
                att_map[p] = att

            if 2 <= it <= P + 1:
                # transpose + copy to SBUF for pair it-2
                p = it - 2
                att = att_map[p]
                att_map[p] = None
                pjt = pjp.tile([128, 384], f32, tag="pj", name=f"pj{p}")
                tp = pjt[:, 0:256].bitcast(bf)  # [128, 512] bf16 staging
                for j in range(2):
                    nc.tensor.transpose(
                        tp[:, 128 * j : 128 * j + 128],
                        att[:, 256 * j : 256 * j + 128],
                        iden_sb[:],
                    )
                    nc.tensor.transpose(
                        tp[:, 256 + 128 * j : 256 + 128 * j + 128],
                        att[:, 256 * j + 128 : 256 * j + 256],
                        iden_sb[:],
                    )
                aT = aTbufs[p % 3]
                nc.vector.tensor_copy(aT[:], tp[:])
                pj_tiles[p] = (pjt, aT)

            if it >= 3:
                # proj + output copy for pair it-3
                p = it - 3
                pjt, aT = pj_tiles[p]
                pj_tiles[p] = None
                for j in range(2):
                    pj = pjt[:, 192 * j : 192 * j + 192]
                    nc.tensor.matmul(
                        pj,
                        aT[:, 128 * j : 128 * j + 128],
                        wt1_sb[:],
                        start=True,
                        stop=False,
                    )
                    nc.tensor.matmul(
                        pj,
                        aT[:, 256 + 128 * j : 256 + 128 * j + 128],
                        wt2_sb[:],
                        start=False,
                        stop=True,
                    )
                ob = obbufs[(p // CHUNK) % 3]
                r = p % CHUNK
                nc.vector.tensor_copy(ob[:, 384 * r : 384 * r + 384], pjt[:])
                if r == CHUNK - 1:
                    nc.gpsimd.dma_start(out=out_d[p // CHUNK], in_=ob[:])

    nc.compile()
    return nc


def _host_prep(qkv, tk_id, proj_w, proj_b):
    """Sort/gather/layout on host. Returns (in_maps, sort_idx)."""
    qkv = np.asarray(qkv, dtype=np.float32)
    tk_id = np.asarray(tk_id)
    proj_w = np.asarray(proj_w, dtype=np.float32)
    proj_b = np.asarray(proj_b, dtype=np.float32)

    sort_idx = np.argsort(tk_id, axis=-1, kind="stable")  # [B, N]
    shuffled = np.take_along_axis(qkv, sort_idx[:, :, None], axis=1)  # [B,N,3C]

    y = shuffled.reshape(B, N // GS, GS, 3, H, DH).reshape(NG_TOTAL, GS, 3, H, DH)
    scale = DH ** (-0.5)
    q = y[:, :, 0] * scale  # [G, t, h, d]
    k = y[:, :, 1]
    v = y[:, :, 2]

    Ptot = NG_TOTAL // 2  # 512
    qp = q.reshape(Ptot, 2, GS, H, DH)  # [p, j, t, h, d]
    kp = k.reshape(Ptot, 2, GS, H, DH)
    qk = np.empty((Ptot, 32, 3072), dtype=BF16)
    for m in range(12):
        j, h = m // 6, m % 6
        qk[:, :, 256 * m : 256 * m + 128] = qp[:, j, :, h, :].transpose(0, 2, 1)
        qk[:, :, 256 * m + 128 : 256 * m + 256] = kp[:, j, :, h, :].transpose(0, 2, 1)

    v1 = np.empty((NG_TOTAL, GS, H, DH + 1), dtype=np.float32)
    v1[..., :DH] = v
    v1[..., DH] = 1.0
    v1 = v1.reshape(NG_TOTAL, GS, H * (DH + 1))  # [G, 128, 198]
    v1p = (
        v1.reshape(Ptot, 2, GS, 198)
        .transpose(0, 2, 1, 3)
        .reshape(Ptot, GS, 396)
        .astype(BF16)
    )

    # proj weights permuted to att-column order (33h+d; d==32 -> bias/6 row)
    wt = proj_w.T.copy()  # [cin, cout]
    b6 = proj_b / 6.0
    wt1 = np.zeros((128, C), np.float32)
    wt2 = np.zeros((128, C), np.float32)
    for r in range(128):
        h, d = r // 33, r % 33
        wt1[r] = b6 if d == 32 else wt[32 * h + d]
    for rp in range(70):
        col = 128 + rp
        h, d = col // 33, col % 33
        wt2[rp] = b6 if d == 32 else wt[32 * h + d]
    wt1 = wt1.astype(BF16)
    wt2 = wt2.astype(BF16)
    iden = np.eye(128, dtype=BF16)

    in_maps = []
    for core in range(N_CORES):
        s = slice(core * PAIRS, (core + 1) * PAIRS)
        qkc = (
            qk[s]
            .reshape(NCHUNK, CHUNK, 32, 3072)
            .transpose(0, 2, 1, 3)
            .reshape(NCHUNK, 32, CHUNK * 3072)
        )
        v1c = (
            v1p[s]
            .reshape(NCHUNK, CHUNK, 128, 396)
            .transpose(0, 2, 1, 3)
            .reshape(NCHUNK, 128, CHUNK * 396)
        )
        in_maps.append(
            {
                "qks": np.ascontiguousarray(qkc),
                "v1": np.ascontiguousarray(v1c),
                "wt1": wt1,
                "wt2": wt2,
                "iden": iden,
            }
        )
    return in_maps, sort_idx


def _host_unshard(results, sort_idx):
    outs = []
    for res in results:
        o = np.asarray(res["out"])  # [NCHUNK, 128, CHUNK*384] bf16
        o = (
            o.reshape(NCHUNK, 128, CHUNK, 384)
            .transpose(0, 2, 1, 3)
            .reshape(PAIRS, 128, 384)
        )
        outs.append(o)
    out_sorted = np.concatenate(outs, axis=0).astype(np.float32)  # [512, 128, 384]
    out_sorted = (
        out_sorted.reshape(NG_TOTAL // 2, GS, 2, C)
        .transpose(0, 2, 1, 3)
        .reshape(B, N, C)
    )
    final = np.empty_like(out_sorted)
    np.put_along_axis(final, sort_idx[:, :, None], out_sorted, axis=1)
    return final


def _get_nc():
    if "nc" not in _nc_cache:
        _nc_cache["nc"] = _build_nc()
    return _nc_cache["nc"]


def _run(in_maps, trace=False):
    from concourse import bass_utils

    nc = _get_nc()
    return bass_utils.run_bass_kernel_spmd(
        nc, in_maps, core_ids=list(range(N_CORES)), trace=trace
    )


def kernel(qkv, tk_id, x_size=None, proj_w=None, proj_b=None):
    in_maps, sort_idx = _host_prep(qkv, tk_id, proj_w, proj_b)
    res = _run(in_maps, trace=False)
    return _host_unshard(res.results, sort_idx)


# revision 25
# speedup vs baseline: 1.5108x; 1.4975x over previous
"""Trainium2 Bass kernel for grouped 128x128 sparse attention + output proj.

Problem (hardcoded): qkv [2, 65536, 576] f32, tk_id [2, 65536] int32 in [0,64),
proj_w [192,192], proj_b [192].  c=192, heads=6, dh=32, group size 128,
ng=512 per batch (no padding since 65536 % 128 == 0).

Strategy:
  - Host: stable argsort by cluster id, gather qkv, reshape to 1024 independent
    groups; data-parallel shard 128 groups/core across 8 NeuronCores.
  - Device, per pair of groups (two groups share the 128-token tiles):
      scores: 12 K=32 matmuls row-tiled 2x via tile_position=(32i,0) into
        4 PSUM banks (two [128,1024] tiles, one per group j)
      exp:    2 scalar activations (FD=768 each) -> SBUF bf16
      AV:     12 matmuls P^T.T @ [v|1] -> av [q, 12*33] f32 (col 33h+32 = denom)
      normalize: vector reciprocal + one broadcast multiply -> att bf16
      transpose: 4 PE transposes into the proj psum bank (bf16 bitcast view)
      proj:   4 matmuls (aT.T @ wt1/wt2, bias via exact-1.0 denom rows)
      out:    vector copy psum -> SBUF bf16, DMA per 4-pair chunk
  - Host: gather per-core outputs, inverse permutation, return [2, 65536, 192].

All DMAs move 4-pair chunks to amortize descriptor-issue cost.
"""

import numpy as np
import ml_dtypes

BF16 = ml_dtypes.bfloat16

B = 2
N = 65536
C = 192
H = 6
DH = 32
GS = 128
NG_TOTAL = B * (N // GS)  # 1024 groups
N_CORES = 8
G_PER_CORE = NG_TOTAL // N_CORES  # 128
PAIRS = G_PER_CORE // 2  # 64
CHUNK = 4  # pairs per DMA chunk
NCHUNK = PAIRS // CHUNK  # 16

_nc_cache = {}


def _build_nc(num_pairs=PAIRS):
    """Build the Bass/Tile graph for one core (SPMD across all 8)."""
    from contextlib import ExitStack

    import concourse.tile as tile
    from concourse import bacc, mybir

    bf = mybir.dt.bfloat16
    f32 = mybir.dt.float32
    EXPF = mybir.ActivationFunctionType.Exp

    nc = bacc.Bacc("TRN2", target_bir_lowering=False, debug=False)

    P = num_pairs
    NCH = P // CHUNK
    # qk: per pair [64 partitions, 1536 cols]; head m=6j+h at (c=m//2, i=m%2):
    #   q_m at partitions 32i:32i+32, cols 256c:256c+128; k_m at +128.
    #   Scores run as K=32 row-tiled matmul pairs (tile_position=(32i,0)).
    qks_d = nc.declare_dram_parameter("qks", [NCH, 64, CHUNK * 1536], bf, isOutput=False)
    # v1: per pair [128, 396]; cols 198j+33h+[v(32)|1].
    v1_d = nc.declare_dram_parameter("v1", [NCH, 128, CHUNK * 396], bf, isOutput=False)
    wt1_d = nc.declare_dram_parameter("wt1", [128, 192], bf, isOutput=False)
    wt2_d = nc.declare_dram_parameter("wt2", [128, 192], bf, isOutput=False)
    iden_d = nc.declare_dram_parameter("iden", [128, 128], bf, isOutput=False)
    out_d = nc.declare_dram_parameter("out", [NCH, 128, CHUNK * 384], bf, isOutput=True)

    with tile.TileContext(nc) as tc, ExitStack() as ctx:
        consts = ctx.enter_context(tc.tile_pool(name="consts", bufs=1))
        wt1_sb = consts.tile([128, 192], bf)
        nc.sync.dma_start(out=wt1_sb[:], in_=wt1_d[:, :])
        wt2_sb = consts.tile([128, 192], bf)
        nc.sync.dma_start(out=wt2_sb[:], in_=wt2_d[:, :])
        iden_sb = consts.tile([128, 128], bf)
        nc.sync.dma_start(out=iden_sb[:], in_=iden_d[:, :])

        qkbufs = [consts.tile([64, CHUNK * 1536], bf, name=f"qkc{i}") for i in range(3)]
        vvbufs = [consts.tile([128, CHUNK * 396], bf, name=f"vvc{i}") for i in range(3)]
        obbufs = [consts.tile([128, CHUNK * 384], bf, name=f"obc{i}") for i in range(3)]
        # att: [q, 256j + 33h + d], cols 198:256 / 454:512 stay zero (pad)
        attbufs = [consts.tile([128, 512], bf, name=f"attb{i}") for i in range(3)]
        for ab in attbufs:
            nc.vector.memset(ab[:, 198:256], 0.0)
            nc.vector.memset(ab[:, 454:512], 0.0)
        aTbufs = [consts.tile([128, 512], bf, name=f"aTb{i}") for i in range(3)]

        expp = ctx.enter_context(tc.tile_pool(name="exps", bufs=4))
        recp = ctx.enter_context(tc.tile_pool(name="rec", bufs=2))
        # PSUM: gt 2x2 banks + av 2 + pj 2 = 8 banks.
        gtp = ctx.enter_context(tc.tile_pool(name="gt", bufs=2, space="PSUM"))
        avp = ctx.enter_context(tc.tile_pool(name="av", bufs=2, space="PSUM"))
        pjp = ctx.enter_context(tc.tile_pool(name="pj", bufs=2, space="PSUM"))

        # chunk 0: per-pair qk slices so pair 0 can start before the rest land
        for r in range(CHUNK):
            nc.sync.dma_start(
                out=qkbufs[0][:, 1536 * r : 1536 * (r + 1)],
                in_=qks_d[0, :, 1536 * r : 1536 * (r + 1)],
            )
        nc.sync.dma_start(out=vvbufs[0][:], in_=v1_d[0])
        nc.sync.dma_start(out=qkbufs[1][:], in_=qks_d[1])
        nc.sync.dma_start(out=vvbufs[1][:], in_=v1_d[1])

        e_tiles = [None] * P
        att_map = [None] * P
        pj_tiles = [None] * P

        for it in range(P + 3):
            # Prefetch 2 chunks ahead into the 3-deep ring: by it%CHUNK==1 the
            # last readers of buffer k%3 (scores/AV of chunk k-3) are already
            # emitted, so the WAR dep is inferred correctly, and the ~5us
            # chunk transfer has two chunk-periods to complete.
            if it < P and it % CHUNK == 1:
                k = it // CHUNK + 2
                if k < NCH:
                    nc.sync.dma_start(out=qkbufs[k % 3][:], in_=qks_d[k])
                    nc.sync.dma_start(out=vvbufs[k % 3][:], in_=v1_d[k])

            if it < P:
                # scores + exp for pair `it`.  high_priority keeps the
                # scores -> EXP feed ahead of older tail work in the static
                # schedule so the scalar engine (the pacing engine) never
                # starves behind a transpose/proj backlog.
                p = it
                qk = qkbufs[(p // CHUNK) % 3]
                qo = 1536 * (p % CHUNK)
                gts = [
                    gtp.tile([128, 1024], f32, tag="gt", name=f"gt{p}_{j}")
                    for j in range(2)
                ]
                with tc.high_priority(offset=200):
                    for c in range(6):
                        for i in range(2):
                            m = 2 * c + i
                            gt = gts[m // 6]
                            cpos = c % 3
                            nc.tensor.matmul(
                                gt[:, 512 * i + 128 * cpos : 512 * i + 128 * cpos + 128],
                                qk[32 * i : 32 * i + 32, qo + 256 * c + 128 : qo + 256 * c + 256],
                                qk[32 * i : 32 * i + 32, qo + 256 * c : qo + 256 * c + 128],
                                start=True,
                                stop=True,
                                tile_position=(32 * i, 0),
                            )
                    ea = expp.tile([128, 768], bf, tag="exp", name=f"ea{p}")
                    eb = expp.tile([128, 768], bf, tag="exp", name=f"eb{p}")
                    nc.scalar.activation(
                        ea[:].rearrange("p (i x) -> p i x", i=2),
                        gts[0][:].rearrange("p (i x) -> p i x", i=2)[:, :, 0:384],
                        EXPF,
                    )
                    nc.scalar.activation(
                        eb[:].rearrange("p (i x) -> p i x", i=2),
                        gts[1][:].rearrange("p (i x) -> p i x", i=2)[:, :, 0:384],
                        EXPF,
                    )
                e_tiles[p] = (ea, eb)

            if 1 <= it <= P:
                # AV + normalize for pair it-1
                p = it - 1
                ea, eb = e_tiles[p]
                e_tiles[p] = None
                vv = vvbufs[(p // CHUNK) % 3]
                vo = 396 * (p % CHUNK)
                av = avp.tile([128, 396], f32, tag="av", name=f"av{p}")
                for m in range(12):
                    j, h = m // 6, m % 6
                    e = ea if j == 0 else eb
                    ecol = 384 * (m % 2) + 128 * ((m // 2) % 3)
                    nc.tensor.matmul(
                        av[:, 198 * j + 33 * h : 198 * j + 33 * h + 33],
                        e[:, ecol : ecol + 128],
                        vv[:, vo + 198 * j + 33 * h : vo + 198 * j + 33 * h + 33],
                        start=True,
                        stop=True,
                    )
                av4 = av[:].rearrange("p (j h x) -> p j h x", j=2, x=33)
                rec = recp.tile([128, 12], f32)
                rec3 = rec[:].rearrange("p (j h) -> p j h", j=2)
                nc.vector.reciprocal(rec3[:, :, :, None], av4[:, :, :, 32:33])
                att = attbufs[p % 3]
                attv = (
                    att[:]
                    .rearrange("p (j x) -> p j x", j=2)[:, :, 0:198]
                    .rearrange("p j (h d) -> p j h d", d=33)
                )
                nc.vector.tensor_mul(
                    attv, av4, rec3[:, :, :, None].to_broadcast((128, 2, 6, 33))
                )
                att_map[p] = att

            if 2 <= it <= P + 1:
                # transpose + copy to SBUF for pair it-2
                p = it - 2
                att = att_map[p]
                att_map[p] = None
                pjt = pjp.tile([128, 384], f32, tag="pj", name=f"pj{p}")
                tp = pjt[:, 0:256].bitcast(bf)  # [128, 512] bf16 staging
                for j in range(2):
                    nc.tensor.transpose(
                        tp[:, 128 * j : 128 * j + 128],
                        att[:, 256 * j : 256 * j + 128],
                        iden_sb[:],
                    )
                    nc.tensor.transpose(
                        tp[:, 256 + 128 * j : 256 + 128 * j + 128],
                        att[:, 256 * j + 128 : 256 * j + 256],
                        iden_sb[:],
                    )
                aT = aTbufs[p % 3]
                nc.vector.tensor_copy(aT[:], tp[:])
                pj_tiles[p] = (pjt, aT)

            if it >= 3:
                # proj + output copy for pair it-3
                p = it - 3
                pjt, aT = pj_tiles[p]
                pj_tiles[p] = None
                for j in range(2):
                    pj = pjt[:, 192 * j : 192 * j + 192]
                    nc.tensor.matmul(
                        pj,
                        aT[:, 128 * j : 128 * j + 128],
                        wt1_sb[:],
                        start=True,
                        stop=False,
                    )
                    nc.tensor.matmul(
                        pj,
                        aT[:, 256 + 128 * j : 256 + 128 * j + 128],
                        wt2_sb[:],
                        start=False,
                        stop=True,
                    )
                ob = obbufs[(p // CHUNK) % 3]
                r = p % CHUNK
                nc.vector.tensor_copy(ob[:, 384 * r : 384 * r + 384], pjt[:])
                if r == CHUNK - 1:
                    nc.gpsimd.dma_start(out=out_d[p // CHUNK], in_=ob[:])

    nc.compile()
    return nc


def _host_prep(qkv, tk_id, proj_w, proj_b):
    """Sort/gather/layout on host. Returns (in_maps, sort_idx)."""
    qkv = np.asarray(qkv, dtype=np.float32)
    tk_id = np.asarray(tk_id)
    proj_w = np.asarray(proj_w, dtype=np.float32)
    proj_b = np.asarray(proj_b, dtype=np.float32)

    sort_idx = np.argsort(tk_id, axis=-1, kind="stable")  # [B, N]
    shuffled = np.take_along_axis(qkv, sort_idx[:, :, None], axis=1)  # [B,N,3C]

    y = shuffled.reshape(B, N // GS, GS, 3, H, DH).reshape(NG_TOTAL, GS, 3, H, DH)
    scale = DH ** (-0.5)
    q = y[:, :, 0] * scale  # [G, t, h, d]
    k = y[:, :, 1]
    v = y[:, :, 2]

    Ptot = NG_TOTAL // 2  # 512
    qp = q.reshape(Ptot, 2, GS, H, DH)  # [p, j, t, h, d]
    kp = k.reshape(Ptot, 2, GS, H, DH)
    qk = np.empty((Ptot, 64, 1536), dtype=BF16)
    for m in range(12):
        j, h = m // 6, m % 6
        c, i = m // 2, m % 2
        qk[:, 32 * i : 32 * i + 32, 256 * c : 256 * c + 128] = qp[
            :, j, :, h, :
        ].transpose(0, 2, 1)
        qk[:, 32 * i : 32 * i + 32, 256 * c + 128 : 256 * c + 256] = kp[
            :, j, :, h, :
        ].transpose(0, 2, 1)

    v1 = np.empty((NG_TOTAL, GS, H, DH + 1), dtype=np.float32)
    v1[..., :DH] = v
    v1[..., DH] = 1.0
    v1 = v1.reshape(NG_TOTAL, GS, H * (DH + 1))  # [G, 128, 198]
    v1p = (
        v1.reshape(Ptot, 2, GS, 198)
        .transpose(0, 2, 1, 3)
        .reshape(Ptot, GS, 396)
        .astype(BF16)
    )

    # proj weights permuted to att-column order (33h+d; d==32 -> bias/6 row)
    wt = proj_w.T.copy()  # [cin, cout]
    b6 = proj_b / 6.0
    wt1 = np.zeros((128, C), np.float32)
    wt2 = np.zeros((128, C), np.float32)
    for r in range(128):
        h, d = r // 33, r % 33
        wt1[r] = b6 if d == 32 else wt[32 * h + d]
    for rp in range(70):
        col = 128 + rp
        h, d = col // 33, col % 33
        wt2[rp] = b6 if d == 32 else wt[32 * h + d]
    wt1 = wt1.astype(BF16)
    wt2 = wt2.astype(BF16)
    iden = np.eye(128, dtype=BF16)

    in_maps = []
    for core in range(N_CORES):
        s = slice(core * PAIRS, (core + 1) * PAIRS)
        qkc = (
            qk[s]
            .reshape(NCHUNK, CHUNK, 64, 1536)
            .transpose(0, 2, 1, 3)
            .reshape(NCHUNK, 64, CHUNK * 1536)
        )
        v1c = (
            v1p[s]
            .reshape(NCHUNK, CHUNK, 128, 396)
            .transpose(0, 2, 1, 3)
            .reshape(NCHUNK, 128, CHUNK * 396)
        )
        in_maps.append(
            {
                "qks": np.ascontiguousarray(qkc),
                "v1": np.ascontiguousarray(v1c),
                "wt1": wt1,
                "wt2": wt2,
                "iden": iden,
            }
        )
    return in_maps, sort_idx


def _host_unshard(results, sort_idx):
    outs = []
    for res in results:
        o = np.asarray(res["out"])  # [NCHUNK, 128, CHUNK*384] bf16
        o = (
            o.reshape(NCHUNK, 128, CHUNK, 384)
            .transpose(0, 2, 1, 3)
            .reshape(PAIRS, 128, 384)
        )
        outs.append(o)
    out_sorted = np.concatenate(outs, axis=0).astype(np.float32)  # [512, 128, 384]
    out_sorted = (
        out_sorted.reshape(NG_TOTAL // 2, GS, 2, C)
        .transpose(0, 2, 1, 3)
        .reshape(B, N, C)
    )
    final = np.empty_like(out_sorted)
    np.put_along_axis(final, sort_idx[:, :, None], out_sorted, axis=1)
    return final


def _get_nc():
    if "nc" not in _nc_cache:
        _nc_cache["nc"] = _build_nc()
    return _nc_cache["nc"]


def _run(in_maps, trace=False):
    from concourse import bass_utils

    nc = _get_nc()
    return bass_utils.run_bass_kernel_spmd(
        nc, in_maps, core_ids=list(range(N_CORES)), trace=trace
    )


def kernel(qkv, tk_id, x_size=None, proj_w=None, proj_b=None):
    in_maps, sort_idx = _host_prep(qkv, tk_id, proj_w, proj_b)
    res = _run(in_maps, trace=False)
    return _host_unshard(res.results, sort_idx)


# revision 26
# speedup vs baseline: 1.5151x; 1.0029x over previous
"""Trainium2 Bass kernel for grouped 128x128 sparse attention + output proj.

Problem (hardcoded): qkv [2, 65536, 576] f32, tk_id [2, 65536] int32 in [0,64),
proj_w [192,192], proj_b [192].  c=192, heads=6, dh=32, group size 128,
ng=512 per batch (no padding since 65536 % 128 == 0).

Strategy:
  - Host: stable argsort by cluster id, gather qkv, reshape to 1024 independent
    groups; data-parallel shard 128 groups/core across 8 NeuronCores.
  - Device, per pair of groups (two groups share the 128-token tiles):
      scores: 12 K=32 matmuls row-tiled 2x via tile_position=(32i,0) into
        4 PSUM banks (two [128,1024] tiles, one per group j)
      exp:    2 scalar activations (FD=768 each) -> SBUF bf16
      AV:     12 matmuls P^T.T @ [v|1] -> av [q, 12*33] f32 (col 33h+32 = denom)
      normalize: vector reciprocal + one broadcast multiply -> att bf16
      transpose: 4 PE transposes into the proj psum bank (bf16 bitcast view)
      proj:   4 matmuls (aT.T @ wt1/wt2, bias via exact-1.0 denom rows)
      out:    vector copy psum -> SBUF bf16, DMA per 4-pair chunk
  - Host: gather per-core outputs, inverse permutation, return [2, 65536, 192].

All DMAs move 4-pair chunks to amortize descriptor-issue cost.
"""

import numpy as np
import ml_dtypes

BF16 = ml_dtypes.bfloat16

B = 2
N = 65536
C = 192
H = 6
DH = 32
GS = 128
NG_TOTAL = B * (N // GS)  # 1024 groups
N_CORES = 8
G_PER_CORE = NG_TOTAL // N_CORES  # 128
PAIRS = G_PER_CORE // 2  # 64
CHUNK = 4  # pairs per DMA chunk
NCHUNK = PAIRS // CHUNK  # 16

_nc_cache = {}


def _build_nc(num_pairs=PAIRS):
    """Build the Bass/Tile graph for one core (SPMD across all 8)."""
    from contextlib import ExitStack

    import concourse.tile as tile
    from concourse import bacc, mybir

    bf = mybir.dt.bfloat16
    f32 = mybir.dt.float32
    EXPF = mybir.ActivationFunctionType.Exp

    nc = bacc.Bacc("TRN2", target_bir_lowering=False, debug=False)

    P = num_pairs
    NCH = P // CHUNK
    # qk: per pair [64 partitions, 1536 cols]; head m=6j+h at (c=m//2, i=m%2):
    #   q_m at partitions 32i:32i+32, cols 256c:256c+128; k_m at +128.
    #   Scores run as K=32 row-tiled matmul pairs (tile_position=(32i,0)).
    qks_d = nc.declare_dram_parameter("qks", [NCH, 64, CHUNK * 1536], bf, isOutput=False)
    # v1: per pair [128, 396]; cols 198j+33h+[v(32)|1].
    v1_d = nc.declare_dram_parameter("v1", [NCH, 128, CHUNK * 396], bf, isOutput=False)
    wt1_d = nc.declare_dram_parameter("wt1", [128, 192], bf, isOutput=False)
    wt2_d = nc.declare_dram_parameter("wt2", [128, 192], bf, isOutput=False)
    iden_d = nc.declare_dram_parameter("iden", [128, 128], bf, isOutput=False)
    out_d = nc.declare_dram_parameter("out", [NCH, 128, CHUNK * 384], bf, isOutput=True)

    with tile.TileContext(nc) as tc, ExitStack() as ctx:
        consts = ctx.enter_context(tc.tile_pool(name="consts", bufs=1))
        wt1_sb = consts.tile([128, 192], bf)
        nc.sync.dma_start(out=wt1_sb[:], in_=wt1_d[:, :])
        wt2_sb = consts.tile([128, 192], bf)
        nc.sync.dma_start(out=wt2_sb[:], in_=wt2_d[:, :])
        iden_sb = consts.tile([128, 128], bf)
        nc.sync.dma_start(out=iden_sb[:], in_=iden_d[:, :])

        qkbufs = [consts.tile([64, CHUNK * 1536], bf, name=f"qkc{i}") for i in range(3)]
        vvbufs = [consts.tile([128, CHUNK * 396], bf, name=f"vvc{i}") for i in range(3)]
        obbufs = [consts.tile([128, CHUNK * 384], bf, name=f"obc{i}") for i in range(3)]
        # att: [q, 256j + 33h + d], cols 198:256 / 454:512 stay zero (pad)
        attbufs = [consts.tile([128, 512], bf, name=f"attb{i}") for i in range(4)]
        for ab in attbufs:
            nc.vector.memset(ab[:, 198:256], 0.0)
            nc.vector.memset(ab[:, 454:512], 0.0)
        aTbufs = [consts.tile([128, 512], bf, name=f"aTb{i}") for i in range(4)]

        expp = ctx.enter_context(tc.tile_pool(name="exps", bufs=6))
        recp = ctx.enter_context(tc.tile_pool(name="rec", bufs=4))
        # PSUM: gt 2x2 banks + av 2 + pj 2 = 8 banks.
        gtp = ctx.enter_context(tc.tile_pool(name="gt", bufs=2, space="PSUM"))
        avp = ctx.enter_context(tc.tile_pool(name="av", bufs=2, space="PSUM"))
        pjp = ctx.enter_context(tc.tile_pool(name="pj", bufs=2, space="PSUM"))

        # chunk 0: per-pair qk slices so pair 0 can start before the rest land
        for r in range(CHUNK):
            nc.sync.dma_start(
                out=qkbufs[0][:, 1536 * r : 1536 * (r + 1)],
                in_=qks_d[0, :, 1536 * r : 1536 * (r + 1)],
            )
        nc.sync.dma_start(out=vvbufs[0][:], in_=v1_d[0])
        nc.sync.dma_start(out=qkbufs[1][:], in_=qks_d[1])
        nc.sync.dma_start(out=vvbufs[1][:], in_=v1_d[1])

        e_tiles = [None] * P
        att_map = [None] * P
        pj_tiles = [None] * P

        for it in range(P + 3):
            # Prefetch 2 chunks ahead into the 3-deep ring: by it%CHUNK==1 the
            # last readers of buffer k%3 (scores/AV of chunk k-3) are already
            # emitted, so the WAR dep is inferred correctly, and the ~5us
            # chunk transfer has two chunk-periods to complete.
            if it < P and it % CHUNK == 1:
                k = it // CHUNK + 2
                if k < NCH:
                    nc.sync.dma_start(out=qkbufs[k % 3][:], in_=qks_d[k])
                    nc.sync.dma_start(out=vvbufs[k % 3][:], in_=v1_d[k])

            if it < P:
                # scores + exp for pair `it`.  high_priority keeps the
                # scores -> EXP feed ahead of older tail work in the static
                # schedule so the scalar engine (the pacing engine) never
                # starves behind a transpose/proj backlog.
                p = it
                qk = qkbufs[(p // CHUNK) % 3]
                qo = 1536 * (p % CHUNK)
                gts = [
                    gtp.tile([128, 1024], f32, tag="gt", name=f"gt{p}_{j}")
                    for j in range(2)
                ]
                with tc.high_priority(offset=200):
                    for c in range(6):
                        for i in range(2):
                            m = 2 * c + i
                            gt = gts[m // 6]
                            cpos = c % 3
                            nc.tensor.matmul(
                                gt[:, 512 * i + 128 * cpos : 512 * i + 128 * cpos + 128],
                                qk[32 * i : 32 * i + 32, qo + 256 * c + 128 : qo + 256 * c + 256],
                                qk[32 * i : 32 * i + 32, qo + 256 * c : qo + 256 * c + 128],
                                start=True,
                                stop=True,
                                tile_position=(32 * i, 0),
                            )
                    ea = expp.tile([128, 768], bf, tag="exp", name=f"ea{p}")
                    eb = expp.tile([128, 768], bf, tag="exp", name=f"eb{p}")
                    nc.scalar.activation(
                        ea[:].rearrange("p (i x) -> p i x", i=2),
                        gts[0][:].rearrange("p (i x) -> p i x", i=2)[:, :, 0:384],
                        EXPF,
                    )
                    nc.scalar.activation(
                        eb[:].rearrange("p (i x) -> p i x", i=2),
                        gts[1][:].rearrange("p (i x) -> p i x", i=2)[:, :, 0:384],
                        EXPF,
                    )
                e_tiles[p] = (ea, eb)

            if 1 <= it <= P:
                # AV + normalize for pair it-1
                p = it - 1
                ea, eb = e_tiles[p]
                e_tiles[p] = None
                vv = vvbufs[(p // CHUNK) % 3]
                vo = 396 * (p % CHUNK)
                av = avp.tile([128, 396], f32, tag="av", name=f"av{p}")
                for m in range(12):
                    j, h = m // 6, m % 6
                    e = ea if j == 0 else eb
                    ecol = 384 * (m % 2) + 128 * ((m // 2) % 3)
                    nc.tensor.matmul(
                        av[:, 198 * j + 33 * h : 198 * j + 33 * h + 33],
                        e[:, ecol : ecol + 128],
                        vv[:, vo + 198 * j + 33 * h : vo + 198 * j + 33 * h + 33],
                        start=True,
                        stop=True,
                    )
                av4 = av[:].rearrange("p (j h x) -> p j h x", j=2, x=33)
                rec = recp.tile([128, 12], f32)
                rec3 = rec[:].rearrange("p (j h) -> p j h", j=2)
                nc.vector.reciprocal(rec3[:, :, :, None], av4[:, :, :, 32:33])
                att = attbufs[p % 4]
                attv = (
                    att[:]
                    .rearrange("p (j x) -> p j x", j=2)[:, :, 0:198]
                    .rearrange("p j (h d) -> p j h d", d=33)
                )
                nc.vector.tensor_mul(
                    attv, av4, rec3[:, :, :, None].to_broadcast((128, 2, 6, 33))
                )
                att_map[p] = att

            if 2 <= it <= P + 1:
                # transpose + copy to SBUF for pair it-2
                p = it - 2
                att = att_map[p]
                att_map[p] = None
                pjt = pjp.tile([128, 384], f32, tag="pj", name=f"pj{p}")
                tp = pjt[:, 0:256].bitcast(bf)  # [128, 512] bf16 staging
                for j in range(2):
                    nc.tensor.transpose(
                        tp[:, 128 * j : 128 * j + 128],
                        att[:, 256 * j : 256 * j + 128],
                        iden_sb[:],
                    )
                    nc.tensor.transpose(
                        tp[:, 256 + 128 * j : 256 + 128 * j + 128],
                        att[:, 256 * j + 128 : 256 * j + 256],
                        iden_sb[:],
                    )
                aT = aTbufs[p % 4]
                nc.vector.tensor_copy(aT[:], tp[:])
                pj_tiles[p] = (pjt, aT)

            if it >= 3:
                # proj + output copy for pair it-3
                p = it - 3
                pjt, aT = pj_tiles[p]
                pj_tiles[p] = None
                for j in range(2):
                    pj = pjt[:, 192 * j : 192 * j + 192]
                    nc.tensor.matmul(
                        pj,
                        aT[:, 128 * j : 128 * j + 128],
                        wt1_sb[:],
                        start=True,
                        stop=False,
                    )
                    nc.tensor.matmul(
                        pj,
                        aT[:, 256 + 128 * j : 256 + 128 * j + 128],
                        wt2_sb[:],
                        start=False,
                        stop=True,
                    )
                ob = obbufs[(p // CHUNK) % 3]
                r = p % CHUNK
                nc.vector.tensor_copy(ob[:, 384 * r : 384 * r + 384], pjt[:])
                if r == CHUNK - 1:
                    nc.gpsimd.dma_start(out=out_d[p // CHUNK], in_=ob[:])

    nc.compile()
    return nc


def _host_prep(qkv, tk_id, proj_w, proj_b):
    """Sort/gather/layout on host. Returns (in_maps, sort_idx)."""
    qkv = np.asarray(qkv, dtype=np.float32)
    tk_id = np.asarray(tk_id)
    proj_w = np.asarray(proj_w, dtype=np.float32)
    proj_b = np.asarray(proj_b, dtype=np.float32)

    sort_idx = np.argsort(tk_id, axis=-1, kind="stable")  # [B, N]
    shuffled = np.take_along_axis(qkv, sort_idx[:, :, None], axis=1)  # [B,N,3C]

    y = shuffled.reshape(B, N // GS, GS, 3, H, DH).reshape(NG_TOTAL, GS, 3, H, DH)
    scale = DH ** (-0.5)
    q = y[:, :, 0] * scale  # [G, t, h, d]
    k = y[:, :, 1]
    v = y[:, :, 2]

    Ptot = NG_TOTAL // 2  # 512
    qp = q.reshape(Ptot, 2, GS, H, DH)  # [p, j, t, h, d]
    kp = k.reshape(Ptot, 2, GS, H, DH)
    qk = np.empty((Ptot, 64, 1536), dtype=BF16)
    for m in range(12):
        j, h = m // 6, m % 6
        c, i = m // 2, m % 2
        qk[:, 32 * i : 32 * i + 32, 256 * c : 256 * c + 128] = qp[
            :, j, :, h, :
        ].transpose(0, 2, 1)
        qk[:, 32 * i : 32 * i + 32, 256 * c + 128 : 256 * c + 256] = kp[
            :, j, :, h, :
        ].transpose(0, 2, 1)

    v1 = np.empty((NG_TOTAL, GS, H, DH + 1), dtype=np.float32)
    v1[..., :DH] = v
    v1[..., DH] = 1.0
    v1 = v1.reshape(NG_TOTAL, GS, H * (DH + 1))  # [G, 128, 198]
    v1p = (
        v1.reshape(Ptot, 2, GS, 198)
        .transpose(0, 2, 1, 3)
        .reshape(Ptot, GS, 396)
        .astype(BF16)
    )

    # proj weights permuted to att-column order (33h+d; d==32 -> bias/6 row)
    wt = proj_w.T.copy()  # [cin, cout]
    b6 = proj_b / 6.0
    wt1 = np.zeros((128, C), np.float32)
    wt2 = np.zeros((128, C), np.float32)
    for r in range(128):
        h, d = r // 33, r % 33
        wt1[r] = b6 if d == 32 else wt[32 * h + d]
    for rp in range(70):
        col = 128 + rp
        h, d = col // 33, col % 33
        wt2[rp] = b6 if d == 32 else wt[32 * h + d]
    wt1 = wt1.astype(BF16)
    wt2 = wt2.astype(BF16)
    iden = np.eye(128, dtype=BF16)

    in_maps = []
    for core in range(N_CORES):
        s = slice(core * PAIRS, (core + 1) * PAIRS)
        qkc = (
            qk[s]
            .reshape(NCHUNK, CHUNK, 64, 1536)
            .transpose(0, 2, 1, 3)
            .reshape(NCHUNK, 64, CHUNK * 1536)
        )
        v1c = (
            v1p[s]
            .reshape(NCHUNK, CHUNK, 128, 396)
            .transpose(0, 2, 1, 3)
            .reshape(NCHUNK, 128, CHUNK * 396)
        )
        in_maps.append(
            {
                "qks": np.ascontiguousarray(qkc),
                "v1": np.ascontiguousarray(v1c),
                "wt1": wt1,
                "wt2": wt2,
                "iden": iden,
            }
        )
    return in_maps, sort_idx


def _host_unshard(results, sort_idx):
    outs = []
    for res in results:
        o = np.asarray(res["out"])  # [NCHUNK, 128, CHUNK*384] bf16
        o = (
            o.reshape(NCHUNK, 128, CHUNK, 384)
            .transpose(0, 2, 1, 3)
            .reshape(PAIRS, 128, 384)
        )
        outs.append(o)
    out_sorted = np.concatenate(outs, axis=0).astype(np.float32)  # [512, 128, 384]
    out_sorted = (
        out_sorted.reshape(NG_TOTAL // 2, GS, 2, C)
        .transpose(0, 2, 1, 3)
        .reshape(B, N, C)
    )
    final = np.empty_like(out_sorted)
    np.put_along_axis(final, sort_idx[:, :, None], out_sorted, axis=1)
    return final


def _get_nc():
    if "nc" not in _nc_cache:
        _nc_cache["nc"] = _build_nc()
    return _nc_cache["nc"]


def _run(in_maps, trace=False):
    from concourse import bass_utils

    nc = _get_nc()
    return bass_utils.run_bass_kernel_spmd(
        nc, in_maps, core_ids=list(range(N_CORES)), trace=trace
    )


def kernel(qkv, tk_id, x_size=None, proj_w=None, proj_b=None):
    in_maps, sort_idx = _host_prep(qkv, tk_id, proj_w, proj_b)
    res = _run(in_maps, trace=False)
    return _host_unshard(res.results, sort_idx)
